# revision 44
# baseline (speedup 1.0000x reference)
"""Trainium2 Bass kernel for nn_Blender (per-style MLP blender).

Strategy
--------
Pure data parallel over the batch: each of the 8 NeuronCores processes
B/8 = 1024 samples with a full replica of the weights. No collectives.

On-chip layout is feature-major ([features -> partitions, batch -> free
dim]); every GEMM contracts along the partition axis with batch (N=512
= one fp32 PSUM bank) as the moving dim.

Key structure vs the straightforward version:
  * The age MLP is exactly rank-1 (its biases are zero and ages >= 0,
    so af = age * c with c = age_w2^T relu(age_w1)). Its fc1
    contribution age[b] * d_s[h] is applied by a DVE
    scalar_tensor_tensor into PSUM, which shrinks fc1 from 6 k-tiles
    to 5 (K = 512 gs + 128 gf2 = 640 = 5*128, 100% PE utilization).
  * Phase 1 (bottleneck MLPs + global MLP) reads a separate fp8e4
    copy of global_styles (and fp8 bn_w1 scaled by 16, undone in the
    ReLU's scale) - 4.7 MB instead of 18.9 MB fp16, so the serial
    DMA-bound prologue shrinks ~4x. Phase 2 streams fp16 gs tiles,
    hidden under tensor-bound fc GEMMs.
  * Output is written fp16 (rel err ~5e-4 vs |y|max; gate is 2e-2).
  * The tensor-engine stream is explicitly interleaved: phase-1 chunk-1
    groups are emitted between the first fc blocks of chunk 0 so the
    (in-order) PE queue never waits on the chunk-1 gs DMA, and fc2
    accumulates k-tile-outer so it never waits on the last y1 ReLU.
"""

import numpy as np

import concourse.bacc as bacc
import concourse.tile as tile
from concourse import mybir
from concourse.bass_utils import run_bass_kernel_spmd

S, D, BN, GH, AH, FCH = 18, 512, 32, 128, 16, 512
B = 8192
N_CORES = 8
BC = B // N_CORES          # samples per core
NB = 512                   # moving-dim (batch) tile = one fp32 PSUM bank
N_CHUNKS = BC // NB
GROUPS = [(0, 4), (4, 4), (8, 4), (12, 4), (16, 2)]
KT1 = 5                    # fc1 k-tiles: 4x gs(128) + gf2(128)
W1C = KT1 * FCH            # wpack columns for fc1
W2C = 16 * 128             # wpack columns for fc2
WSC = 16.0                 # bn_w1 fp8 pre-scale

F32 = mybir.dt.float32
MM_DT = mybir.dt.float16
F8 = mybir.dt.float8e4
NP_MM = np.float16

_CACHE = {}


def build_program(debug_taps=False):
    nc = bacc.Bacc("TRN2", target_bir_lowering=False, debug=False,
                   num_devices=N_CORES)
    mm = nc.tensor.matmul

    def din(name, shape, dt=MM_DT):
        return nc.dram_tensor(name, shape, dt, kind="ExternalInput").ap()

    nG = len(GROUPS)
    # pre-chunked, per-partition-contiguous layouts (host-transposed):
    # gsT8c[c, s, p, kt*NB+b] = gs[s, kt*128+p, c*NB+b] in fp8; same for fp16
    gsT8c = din("gsT8c", [N_CHUNKS, S, 128, 4 * NB], F8)
    gsT16c = din("gsT16c", [N_CHUNKS, S, 128, 4 * NB])
    aT = din("aT", [128, BC])                      # ages broadcast to 128 parts
    fcc32 = din("fcc32", [128, 3 * S * 4], F32)    # dT | b1T | b2T
    bn_w1t = din("bn_w1t", [128, S * 4 * BN], F8)
    bnc16 = din("bnc16", [128, nG * 128 + nG * GH + GH])  # bn_w2bd|gm_w1g|gm_w2
    bnc32 = din("bnc32", [128, 2 * nG + 2], F32)   # bn_b1g|bn_b2g|gm_b1|gm_b2
    wpack = din("wpack", [S, 128, W1C + W2C])      # fc1 | fc2 weights per style
    yT = nc.dram_tensor("yT", [S, D, BC], MM_DT, kind="ExternalOutput").ap()
    if debug_taps:
        dbg = {
            "dbg_gf2": nc.dram_tensor("dbg_gf2", [N_CHUNKS, GH, NB], MM_DT,
                                      kind="ExternalOutput").ap(),
            "dbg_h1": nc.dram_tensor("dbg_h1", [128, NB], MM_DT,
                                     kind="ExternalOutput").ap(),
            "dbg_a": nc.dram_tensor("dbg_a", [128, BC], MM_DT,
                                    kind="ExternalOutput").ap(),
            "dbg_d": nc.dram_tensor("dbg_d", [128, S * 4], F32,
                                    kind="ExternalOutput").ap(),
            "dbg_w": nc.dram_tensor("dbg_w", [128, 512], MM_DT,
                                    kind="ExternalOutput").ap(),
            "dbg_gs16": nc.dram_tensor("dbg_gs16", [128, 512], MM_DT,
                                       kind="ExternalOutput").ap(),
            "dbg_gs8": nc.dram_tensor("dbg_gs8", [128, 512], F8,
                                      kind="ExternalOutput").ap(),
            "dbg_y1": nc.dram_tensor("dbg_y1", [128, NB], MM_DT,
                                     kind="ExternalOutput").ap(),
        }

    Relu = mybir.ActivationFunctionType.Relu
    Ident = mybir.ActivationFunctionType.Identity
    ADD = mybir.AluOpType.add
    MULT = mybir.AluOpType.mult

    with (
        tile.TileContext(nc) as tc,
        tc.tile_pool(name="consts", bufs=1) as consts,
        tc.tile_pool(name="gs8p", bufs=1) as gs8_pool,        # 18 c1 tiles live at once
        tc.tile_pool(name="gs16p", bufs=1) as gs16_pool,
        tc.tile_pool(name="act1", bufs=1) as act1_pool,
        tc.tile_pool(name="wp", bufs=1) as w_pool,
        tc.tile_pool(name="y1p", bufs=2) as y1_pool,
        tc.tile_pool(name="outp", bufs=4) as out_pool,
        tc.tile_pool(name="ps", bufs=1, space="PSUM") as ps,
    ):
        # ---- resident constants (3 packed DMAs for fast startup) ----
        bn_w1_sb = consts.tile([128, S * 4 * BN], F8, tag="bn_w1")
        nc.sync.dma_start(bn_w1_sb[:], bn_w1t[:])
        bnc16_sb = consts.tile([128, nG * 128 + nG * GH + GH], MM_DT, tag="bnc16")
        bnc32_sb = consts.tile([128, 2 * nG + 2], F32, tag="bnc32")

        def load_bn_consts():
            nc.sync.dma_start(bnc32_sb[:], bnc32[:])
            nc.sync.dma_start(bnc16_sb[:], bnc16[:])
        bn_w2_sb = bnc16_sb[:, :nG * 128]
        gm_w1_sb = bnc16_sb[:, nG * 128:nG * 128 + nG * GH]
        gm_w2_sb = bnc16_sb[:, nG * 128 + nG * GH:]
        bn_b1_sb = bnc32_sb[:, :nG]
        bn_b2_sb = bnc32_sb[:, nG:2 * nG]
        gm_b1_sb = bnc32_sb[:, 2 * nG:2 * nG + 1]
        gm_b2_sb = bnc32_sb[:, 2 * nG + 1:]
        # fc-phase consts: allocate now, DMA deferred until after the gs8
        # chunk-0 stream so phase 1 starts sooner
        a_sb = consts.tile([128, BC], MM_DT, tag="aT")
        fcc_sb = consts.tile([128, 3 * S * 4], F32, tag="fcc32")
        d_sb = fcc_sb[:, :S * 4]
        b1_sb = fcc_sb[:, S * 4:2 * S * 4]
        b2_sb = fcc_sb[:, 2 * S * 4:]

        def load_fc_consts():
            nc.sync.dma_start(a_sb[:], aT[:])
            nc.sync.dma_start(fcc_sb[:], fcc32[:])

        gf2_sb = [consts.tile([GH, NB], MM_DT, tag=f"gf2c{c}", name=f"gf2c{c}")
                  for c in range(N_CHUNKS)]

        # PSUM tags: y1(2) + yo(4) + p1tmp(2) = 8 banks.

        def load_gs8_group(gi, c):
            """one DMA for a whole style group's fp8 gs chunk (ng segments)."""
            s0, ng = GROUPS[gi]
            t = gs8_pool.tile([128, ng * 4 * NB], F8,
                              tag="gs8a" if ng == 4 else "gs8b",
                              bufs=5 if ng == 4 else 2,
                              name=f"gs8_{gi}_{c}")
            nc.sync.dma_start(
                t[:].rearrange("p (s x) -> p s x", s=ng),
                gsT8c[c, s0:s0 + ng].rearrange("s p x -> p s x"))
            return t

        gf_tiles = {}      # (gi, c) -> SBUF gf tile

        def p1_bn1(gi, c, t, ps_tag):
            """bn first layer for one style group -> PSUM h1 accumulator."""
            s0, ng = GROUPS[gi]
            ps_h1 = ps.tile([128, NB], F32, tag=ps_tag,
                            bufs=(2 if ps_tag in ("p1tmp", "y1") else 1),
                            name=f"ps_h1_{gi}_{c}")
            for j in range(ng):
                for kt in range(4):
                    mm(ps_h1[32 * j:32 * j + 32, :],
                       bn_w1_sb[:, ((s0 + j) * 4 + kt) * BN:((s0 + j) * 4 + kt + 1) * BN],
                       t[:, (j * 4 + kt) * NB:(j * 4 + kt + 1) * NB],
                       start=(kt == 0), stop=(kt == 3),
                       tile_position=(0, 32 * j))
            return ps_h1

        def p1_h1act(gi, c, ps_h1, bufs=5):
            s0, ng = GROUPS[gi]
            pN = 32 * ng
            h1 = act1_pool.tile([128, NB], MM_DT, tag="h1s", bufs=bufs,
                                name=f"h1_{gi}_{c}")
            nc.scalar.activation(h1[:pN, :], ps_h1[:pN, :], Relu,
                                 bias=bn_b1_sb[:pN, gi:gi + 1], scale=1.0 / WSC)
            return h1

        def p1_bn2(gi, c, h1):
            s0, ng = GROUPS[gi]
            pN = 32 * ng
            ps_h2 = ps.tile([128, NB], F32, tag="p1tmp", bufs=2,
                            name=f"ps_h2_{gi}_{c}")
            mm(ps_h2[:pN, :], bn_w2_sb[:pN, gi * 128:gi * 128 + pN], h1[:pN, :])
            gf = act1_pool.tile([128, NB], MM_DT, tag=f"gf{gi}", bufs=2,
                                name=f"gf_{gi}_{c}")
            nc.scalar.activation(gf[:pN, :], ps_h2[:pN, :], Ident,
                                 bias=bn_b2_sb[:pN, gi:gi + 1])
            gf_tiles[(gi, c)] = gf

        def p1_group(gi, c, t):
            """serial-chain variant used for chunk 1 (hidden under fc work)."""
            ps_h1 = p1_bn1(gi, c, t, "p1tmp")
            if debug_taps and gi == 0 and c == 1:
                nc.gpsimd.dma_start(dbg["dbg_gs8"][:], t[:, :512])
            h1 = p1_h1act(gi, c, ps_h1)
            p1_bn2(gi, c, h1)

        def p1_gm(c):
            """global MLP for chunk c from the 5 gf tiles -> gf2_sb[c]."""
            ps_g1 = ps.tile([GH, NB], F32, tag="p1tmp", bufs=2, name=f"ps_g1_{c}")
            for gi, (s0, ng) in enumerate(GROUPS):
                pN = 32 * ng
                mm(ps_g1[:], gm_w1_sb[:pN, gi * GH:(gi + 1) * GH],
                   gf_tiles[(gi, c)][:pN, :],
                   start=(gi == 0), stop=(gi == len(GROUPS) - 1))
            gmh = act1_pool.tile([GH, NB], MM_DT, tag="gmh", bufs=2,
                                 name=f"gmh_{c}")
            nc.scalar.activation(gmh[:], ps_g1[:], Relu, bias=gm_b1_sb)
            ps_g2 = ps.tile([GH, NB], F32, tag="p1tmp", bufs=2, name=f"ps_g2_{c}")
            mm(ps_g2[:], gm_w2_sb, gmh[:])
            nc.scalar.activation(gf2_sb[c][:], ps_g2[:], Ident, bias=gm_b2_sb)
            if debug_taps:
                nc.gpsimd.dma_start(dbg["dbg_gf2"][c], gf2_sb[c][:])

        w_tiles = {}
        gs16_tiles = {}

        def pf_w(s):
            w = w_pool.tile([128, W1C + W2C], MM_DT, tag="w", bufs=7,
                            name=f"w_{s}")
            # two DMAs: >8KB per partition in one descriptor corrupts
            nc.sync.dma_start(w[:, :W1C], wpack[s, :, :W1C])
            nc.sync.dma_start(w[:, W1C:], wpack[s, :, W1C:])
            w_tiles[s] = w

        def pf_gs16(s, c):
            t = gs16_pool.tile([128, 4 * NB], MM_DT, tag="gs16", bufs=12,
                               name=f"gs16_{s}_{c}")
            nc.sync.dma_start(t[:], gsT16c[c, s])
            gs16_tiles[(s, c)] = t

        pending = []     # deferred epilogue of the previous fc block

        def flush_epilogue():
            while pending:
                ps_y, s, c, gs_sb, b0 = pending.pop(0)
                for dt_ in range(4):
                    o = out_pool.tile([128, NB], MM_DT, tag="o",
                                      name=f"o_{s}_{c}_{dt_}")
                    nc.vector.scalar_tensor_tensor(
                        o[:], ps_y[dt_][:], b2_sb[:, s * 4 + dt_:s * 4 + dt_ + 1],
                        gs_sb[:, dt_ * NB:(dt_ + 1) * NB], op0=ADD, op1=ADD)
                    nc.gpsimd.dma_start(
                        yT[s, dt_ * 128:(dt_ + 1) * 128, b0:b0 + NB], o[:])

        def fc_block(s, c):
            b0 = c * NB
            w = w_tiles[s]
            gs_sb = gs16_tiles.pop((s, c))
            y1 = []
            for ht in range(4):
                h0 = ht * 128
                ps_y1 = ps.tile([128, NB], F32, tag="y1", bufs=2,
                                name=f"ps_y1_{s}_{c}_{ht}")
                for kt in range(KT1):
                    mov = (gs_sb[:, kt * NB:(kt + 1) * NB] if kt < 4
                           else gf2_sb[c][:])
                    mm(ps_y1[:], w[:, kt * FCH + h0:kt * FCH + h0 + 128], mov,
                       start=(kt == 0), stop=(kt == KT1 - 1))
                # += age * d_s (rank-1 folded age-MLP contribution); stage in
                # SBUF so the DVE never writes PSUM (PE write-port contention)
                y1p = y1_pool.tile([128, NB], MM_DT, tag="y1pre", bufs=3,
                                   name=f"y1p_{s}_{c}_{ht}")
                nc.vector.scalar_tensor_tensor(
                    y1p[:], a_sb[:, b0:b0 + NB], d_sb[:, s * 4 + ht:s * 4 + ht + 1],
                    ps_y1[:], op0=MULT, op1=ADD)
                y1t = y1_pool.tile([128, NB], MM_DT, tag=f"y1_{ht}",
                                   name=f"y1_{s}_{c}_{ht}")
                nc.scalar.activation(y1t[:], y1p[:], Relu,
                                     bias=b1_sb[:, s * 4 + ht:s * 4 + ht + 1])
                y1.append(y1t)
                if debug_taps and s == 0 and c == 0 and ht == 0:
                    nc.gpsimd.dma_start(dbg["dbg_y1"][:], y1t[:])
                    nc.gpsimd.dma_start(dbg["dbg_a"][:], a_sb[:])
                    nc.gpsimd.dma_start(dbg["dbg_w"][:], w[:, :512])
                    nc.gpsimd.dma_start(dbg["dbg_gs16"][:], gs_sb[:, :512])
            # previous block's epilogue runs on the DVE during this block's
            # fc1 so the y1-stt chain is never queued behind it
            flush_epilogue()
            ps_y = [ps.tile([128, NB], F32, tag=f"yo{dt_}", name=f"ps_y_{s}_{c}_{dt_}")
                    for dt_ in range(4)]
            for kt in range(4):          # k-tile outer: no wait on the last ReLU
                for dt_ in range(4):
                    mm(ps_y[dt_][:],
                       w[:, W1C + (kt * 4 + dt_) * 128:W1C + (kt * 4 + dt_ + 1) * 128],
                       y1[kt][:],
                       start=(kt == 0), stop=(kt == 3))
            pending.append((ps_y, s, c, gs_sb, b0))

        # ---------------- emission ----------------
        # phase 1 chunk 0, cross-group pipelined: all bn1 GEMMs first
        # (borrowing the idle fc PSUM banks). ACT-queue order interleaves the
        # h1-ReLUs with the gf-ACTs so no engine's in-order queue waits on a
        # later DMA-paced group.
        p1c0_tags = ["y1", "yo0", "yo1", "yo2", "yo3"]
        ps_h1s = [None] * len(GROUPS)
        h1s = [None] * len(GROUPS)
        t0 = load_gs8_group(0, 0)
        t1 = load_gs8_group(1, 0)
        load_bn_consts()
        # HAM warmup: ~10 back-to-back matmuls into a scratch PSUM bank as
        # soon as the first weight tile lands; sustains the PE-busy window so
        # the clock is at 2.4 GHz before real phase-1 work, which otherwise
        # runs entirely in the cold 1.2 GHz state (it is DMA-paced and never
        # stays busy long enough to warm up on its own)
        warm_ps = ps.tile([128, NB], F32, tag="yo3", name="warm_ps")

        def warm(n):
            # dependency-free matmuls: run only while real work waits on DMA,
            # keeping the HAM clock at 2.4 GHz through the DMA-paced prologue.
            # All warm mms must be emitted before group 4's p1_bn1 (it reuses
            # the yo3 bank warm_ps writes to).
            for _ in range(n):
                mm(warm_ps[:], bn_w1_sb[:, :128], bn_w1_sb[:, :NB])

        # NOTE: more warmup (e.g. filling every phase-1 DMA gap with dummies)
        # trips the chip's sustained-power P0 downclock — the whole run then
        # executes at 2.0 GHz instead of 2.4 (measured 408us vs 338us).
        warm(10)
        ps_h1s[0] = p1_bn1(0, 0, t0, p1c0_tags[0])
        ps_h1s[1] = p1_bn1(1, 0, t1, p1c0_tags[1])
        h1s[0] = p1_h1act(0, 0, ps_h1s[0], bufs=5)
        for gi in range(2, len(GROUPS)):
            t = load_gs8_group(gi, 0)
            ps_h1s[gi] = p1_bn1(gi, 0, t, p1c0_tags[gi])
            h1s[gi - 1] = p1_h1act(gi - 1, 0, ps_h1s[gi - 1], bufs=5)
            p1_bn2(gi - 2, 0, h1s[gi - 2])
        h1s[-1] = p1_h1act(len(GROUPS) - 1, 0, ps_h1s[-1], bufs=5)
        for gi in range(len(GROUPS) - 2, len(GROUPS)):
            p1_bn2(gi, 0, h1s[gi])
        p1_gm(0)
        load_fc_consts()

        # start region: interleave fc group-0 deps with the chunk-1 gs8
        # stream; all of these DMA issues are wait-free so arrival order on
        # the sync queue == issue order == need order.
        gs8_c1 = {}
        for gi in range(len(GROUPS)):
            if gi < 4:
                pf_w(gi)
                pf_gs16(gi, 0)
            gs8_c1[gi] = load_gs8_group(gi, 1)

        # fc chunk 0 of group 0, interleaved with phase-1 chunk 1 in stages
        # (bn1 / ReLU emitted one fc block before bn2 so the in-order tensor
        # queue never waits on an ACT chain)
        pc1 = {}
        hc1 = {}

        def p1c1_ab(gi):
            pc1[gi] = p1_bn1(gi, 1, gs8_c1[gi], "p1tmp")
            hc1[gi] = p1_h1act(gi, 1, pc1[gi])

        fc_block(0, 0)
        p1c1_ab(0)
        pf_gs16(4, 0)
        pf_w(4)
        fc_block(1, 0)
        p1_bn2(0, 1, hc1[0])
        p1c1_ab(1)
        pf_gs16(5, 0)
        pf_w(5)
        fc_block(2, 0)
        p1_bn2(1, 1, hc1[1])
        p1c1_ab(2)
        # group-0 chunk-1 tiles are not needed until after fc(5,0); issuing
        # them here keeps the saturated startup DMA window free for the
        # pulled-forward fc(4,0)/fc(5,0) dependencies
        for s in range(4):
            pf_gs16(s, 1)
        fc_block(3, 0)
        p1_bn2(2, 1, hc1[2])
        p1c1_ab(3)
        # two group-1 chunk-0 blocks pulled forward so the phase-1 chunk-1
        # tail (g3/g4 chains + global MLP) hides under fc work instead of
        # stalling fc(0,1) on the gf2[1] ACT chain
        fc_block(4, 0)
        p1_bn2(3, 1, hc1[3])
        p1c1_ab(4)
        fc_block(5, 0)
        p1_bn2(4, 1, hc1[4])
        p1_gm(1)

        # fc chunk 1 of group 0, prefetching the rest of group 1
        pf_gs16(6, 0)
        pf_gs16(7, 0)
        pf_w(6)
        fc_block(0, 1)
        pf_w(7)
        fc_block(1, 1)
        for s in range(4, 8):
            pf_gs16(s, 1)
        # group 2's chunk-0 deps go out early: group 1's own chunk-0 list is
        # only two blocks long, too late for a just-in-time prefetch there
        for s in range(8, 12):
            pf_gs16(s, 0)
        pf_w(8)
        fc_block(2, 1)
        fc_block(3, 1)

        # steady groups: chunk 0 then chunk 1, prefetching group g+1 at
        # points where the buffer rotation never blocks the sync queue
        for g in range(1, len(GROUPS)):
            s0, ng = GROUPS[g]
            nxt = GROUPS[g + 1] if g + 1 < len(GROUPS) else None
            c0_styles = [6, 7] if g == 1 else list(range(s0, s0 + ng))
            for bi, s in enumerate(c0_styles):
                if bi == 0 and nxt:
                    if g == 1:
                        pf_w(9)      # rest of group 2 prefetched earlier
                    else:
                        for s2 in range(nxt[0], nxt[0] + nxt[1]):
                            pf_gs16(s2, 0)
                        for s2 in range(nxt[0], min(nxt[0] + 2, nxt[0] + nxt[1])):
                            pf_w(s2)
                fc_block(s, 0)
            for bi, s in enumerate(range(s0, s0 + ng)):
                if bi == 0 and nxt:
                    for s2 in range(nxt[0], nxt[0] + nxt[1]):
                        pf_gs16(s2, 1)
                if bi == 1 and nxt:
                    for s2 in range(nxt[0] + 2, nxt[0] + nxt[1]):
                        pf_w(s2)
                fc_block(s, 1)
        flush_epilogue()

    nc.compile()
    return nc


def _prep_weights(bn_w1, bn_b1, bn_w2, bn_b2, gm_w1, gm_b1, gm_w2, gm_b2,
                  age_w1, age_w2, fc_w1, fc_b1, fc_w2, fc_b2):
    import ml_dtypes
    f = np.float32
    h = NP_MM
    f8 = ml_dtypes.float8_e4m3
    nG = len(GROUPS)
    # [p, (s, kt, j)] : bn_w1[s, kt*128+p, j], pre-scaled for fp8
    bn_w1t = np.ascontiguousarray(
        (bn_w1 * WSC).reshape(S, 4, 128, BN).transpose(2, 0, 1, 3)
        .reshape(128, S * 4 * BN).astype(f8))
    bn_b1g = np.zeros((128, nG), f)
    bn_b2g = np.zeros((128, nG), f)
    bn_w2bd = np.zeros((128, nG * 128), h)
    for gi, (s0, ng) in enumerate(GROUPS):
        for j in range(ng):
            bn_b1g[32 * j:32 * j + 32, gi] = bn_b1[s0 + j]
            bn_b2g[32 * j:32 * j + 32, gi] = bn_b2[s0 + j]
            bn_w2bd[32 * j:32 * j + 32, gi * 128 + 32 * j:gi * 128 + 32 * j + 32] = bn_w2[s0 + j]
    gm_w1p = np.zeros((nG * 128, GH), f)
    gm_w1p[:S * BN] = gm_w1
    gm_w1g = np.ascontiguousarray(
        gm_w1p.reshape(nG, 128, GH).transpose(1, 0, 2).reshape(128, nG * GH), h)
    # fc1 rows reordered to [gs (512) | gf (128)]; age handled as rank-1
    w1p = np.empty((S, KT1 * 128, FCH), h)
    w1p[:, :4 * 128] = fc_w1[:, GH + AH:]
    w1p[:, 4 * 128:] = fc_w1[:, :GH]
    fc_w1t = w1p.reshape(S, KT1, 128, FCH).transpose(0, 2, 1, 3).reshape(S, 128, W1C)
    fc_w2t = fc_w2.reshape(S, 4, 128, 4, 128).transpose(0, 2, 1, 3, 4).reshape(S, 128, W2C).astype(h)
    wpack = np.ascontiguousarray(np.concatenate([fc_w1t, fc_w2t], axis=2))
    # rank-1 age direction: d[s, h] = sum_k relu(age_w1)[0,k'] age_w2[k',k] fc_w1[s, GH+k, h]
    c16 = (np.maximum(age_w1[0].astype(np.float64), 0) @ age_w2.astype(np.float64))
    d = np.einsum('k,skh->sh', c16, fc_w1[:, GH:GH + AH, :].astype(np.float64))
    dT = d.reshape(S, 4, 128).transpose(2, 0, 1).reshape(128, S * 4)
    b1T = fc_b1.reshape(S, 4, 128).transpose(2, 0, 1).reshape(128, S * 4)
    b2T = fc_b2.reshape(S, 4, 128).transpose(2, 0, 1).reshape(128, S * 4)
    fcc32 = np.ascontiguousarray(
        np.concatenate([dT, b1T, b2T], axis=1).astype(f))
    bnc16 = np.ascontiguousarray(
        np.concatenate([bn_w2bd, gm_w1g, gm_w2.astype(h)], axis=1))
    bnc32 = np.ascontiguousarray(np.concatenate(
        [bn_b1g, bn_b2g, gm_b1.reshape(GH, 1).astype(f),
         gm_b2.reshape(GH, 1).astype(f)], axis=1).astype(f))
    return dict(bn_w1t=bn_w1t, bnc16=bnc16, bnc32=bnc32, wpack=wpack,
                fcc32=fcc32)


def run(inputs: dict, trace: bool = False):
    """Build in_maps from full inputs, run SPMD on 8 cores, return
    (full_output, BassKernelResults)."""
    import ml_dtypes
    if "nc" not in _CACHE:
        _CACHE["nc"] = build_program()
    nc = _CACHE["nc"]

    gs = inputs["global_styles"]
    ages = inputs["target_ages"]
    w = _prep_weights(
        inputs["bn_w1"], inputs["bn_b1"], inputs["bn_w2"], inputs["bn_b2"],
        inputs["gm_w1"], inputs["gm_b1"], inputs["gm_w2"], inputs["gm_b2"],
        inputs["age_w1"], inputs["age_w2"],
        inputs["fc_w1"], inputs["fc_b1"], inputs["fc_w2"], inputs["fc_b2"])

    gsT_full = np.ascontiguousarray(gs.transpose(1, 2, 0).astype(NP_MM))  # [S, D, B]
    a16 = ages.astype(NP_MM)
    in_maps = []
    for core in range(N_CORES):
        sl = slice(core * BC, (core + 1) * BC)
        # pre-chunked contiguous layout: [c, s, p, kt*NB+b]
        g16c = np.ascontiguousarray(
            gsT_full[:, :, sl].reshape(S, 4, 128, N_CHUNKS, NB)
            .transpose(3, 0, 2, 1, 4).reshape(N_CHUNKS, S, 128, 4 * NB))
        m = dict(w)
        m["gsT16c"] = g16c
        m["gsT8c"] = g16c.astype(ml_dtypes.float8_e4m3)
        m["aT"] = np.ascontiguousarray(np.broadcast_to(a16[None, sl], (128, BC)))
        in_maps.append(m)

    res = run_bass_kernel_spmd(nc, in_maps, core_ids=list(range(N_CORES)),
                               trace=trace)
    yT = np.concatenate([res.results[c]["yT"][:, :, :] for c in range(N_CORES)],
                        axis=2)                              # [S, D, B] fp16
    y = np.ascontiguousarray(yT.transpose(2, 0, 1), np.float32)  # [B, S, D]
    return y, res


def kernel(**inputs) -> np.ndarray:
    y, _ = run(inputs, trace=False)
    return y


# revision 49
# speedup vs baseline: 1.0186x; 1.0186x over previous
"""Trainium2 Bass kernel for nn_Blender (per-style MLP blender).

Strategy
--------
Pure data parallel over the batch: each of the 8 NeuronCores processes
B/8 = 1024 samples with a full replica of the weights. No collectives.

On-chip layout is feature-major ([features -> partitions, batch -> free
dim]); every GEMM contracts along the partition axis with batch (N=512
= one fp32 PSUM bank) as the moving dim.

Key structure vs the straightforward version:
  * The age MLP is exactly rank-1 (its biases are zero and ages >= 0,
    so af = age * c with c = age_w2^T relu(age_w1)). Its fc1
    contribution age[b] * d_s[h] is applied by a DVE
    scalar_tensor_tensor into PSUM, which shrinks fc1 from 6 k-tiles
    to 5 (K = 512 gs + 128 gf2 = 640 = 5*128, 100% PE utilization).
  * Phase 1 (bottleneck MLPs + global MLP) reads a separate fp8e4
    copy of global_styles (and fp8 bn_w1 scaled by 16, undone in the
    ReLU's scale) - 4.7 MB instead of 18.9 MB fp16, so the serial
    DMA-bound prologue shrinks ~4x. Phase 2 streams fp16 gs tiles,
    hidden under tensor-bound fc GEMMs.
  * Output is written fp16 (rel err ~5e-4 vs |y|max; gate is 2e-2).
  * The tensor-engine stream is explicitly interleaved: phase-1 chunk-1
    groups are emitted between the first fc blocks of chunk 0 so the
    (in-order) PE queue never waits on the chunk-1 gs DMA, and fc2
    accumulates k-tile-outer so it never waits on the last y1 ReLU.
"""

import numpy as np

import concourse.bacc as bacc
import concourse.tile as tile
from concourse import mybir
from concourse.bass_utils import run_bass_kernel_spmd

S, D, BN, GH, AH, FCH = 18, 512, 32, 128, 16, 512
B = 8192
N_CORES = 8
BC = B // N_CORES          # samples per core
NB = 512                   # moving-dim (batch) tile = one fp32 PSUM bank
N_CHUNKS = BC // NB
GROUPS = [(0, 4), (4, 4), (8, 4), (12, 4), (16, 2)]
KT1 = 5                    # fc1 k-tiles: 4x gs(128) + gf2(128)
W1C = KT1 * FCH            # wpack columns for fc1
W2C = 16 * 128             # wpack columns for fc2
WSC = 16.0                 # bn_w1 fp8 pre-scale

F32 = mybir.dt.float32
MM_DT = mybir.dt.float16
F8 = mybir.dt.float8e4
NP_MM = np.float16

_CACHE = {}


def build_program(debug_taps=False):
    nc = bacc.Bacc("TRN2", target_bir_lowering=False, debug=False,
                   num_devices=N_CORES)
    mm = nc.tensor.matmul

    def din(name, shape, dt=MM_DT):
        return nc.dram_tensor(name, shape, dt, kind="ExternalInput").ap()

    nG = len(GROUPS)
    # pre-chunked, per-partition-contiguous layouts (host-transposed):
    # gsT8c[c, s, p, kt*NB+b] = gs[s, kt*128+p, c*NB+b] in fp8; same for fp16
    gsT8c = din("gsT8c", [N_CHUNKS, S, 128, 4 * NB], F8)
    gsT16c = din("gsT16c", [N_CHUNKS, S, 128, 4 * NB])
    aT = din("aT", [128, BC])                      # ages broadcast to 128 parts
    fcc32 = din("fcc32", [128, 3 * S * 4], F32)    # dT | b1T | b2T
    bn_w1t = din("bn_w1t", [128, S * 4 * BN], F8)
    bnc16 = din("bnc16", [128, nG * 128 + nG * GH + GH])  # bn_w2bd|gm_w1g|gm_w2
    bnc32 = din("bnc32", [128, 2 * nG + 2], F32)   # bn_b1g|bn_b2g|gm_b1|gm_b2
    wpack = din("wpack", [S, 128, W1C + W2C])      # fc1 | fc2 weights per style
    yT = nc.dram_tensor("yT", [S, D, BC], MM_DT, kind="ExternalOutput").ap()
    if debug_taps:
        dbg = {
            "dbg_gf2": nc.dram_tensor("dbg_gf2", [N_CHUNKS, GH, NB], MM_DT,
                                      kind="ExternalOutput").ap(),
            "dbg_h1": nc.dram_tensor("dbg_h1", [128, NB], MM_DT,
                                     kind="ExternalOutput").ap(),
            "dbg_a": nc.dram_tensor("dbg_a", [128, BC], MM_DT,
                                    kind="ExternalOutput").ap(),
            "dbg_d": nc.dram_tensor("dbg_d", [128, S * 4], F32,
                                    kind="ExternalOutput").ap(),
            "dbg_w": nc.dram_tensor("dbg_w", [128, 512], MM_DT,
                                    kind="ExternalOutput").ap(),
            "dbg_gs16": nc.dram_tensor("dbg_gs16", [128, 512], MM_DT,
                                       kind="ExternalOutput").ap(),
            "dbg_gs8": nc.dram_tensor("dbg_gs8", [128, 512], F8,
                                      kind="ExternalOutput").ap(),
            "dbg_y1": nc.dram_tensor("dbg_y1", [128, NB], MM_DT,
                                     kind="ExternalOutput").ap(),
        }

    Relu = mybir.ActivationFunctionType.Relu
    Ident = mybir.ActivationFunctionType.Identity
    ADD = mybir.AluOpType.add
    MULT = mybir.AluOpType.mult

    with (
        tile.TileContext(nc) as tc,
        tc.tile_pool(name="consts", bufs=1) as consts,
        tc.tile_pool(name="gs8p", bufs=1) as gs8_pool,        # 18 c1 tiles live at once
        tc.tile_pool(name="gs16p", bufs=1) as gs16_pool,
        tc.tile_pool(name="act1", bufs=1) as act1_pool,
        tc.tile_pool(name="wp", bufs=1) as w_pool,
        tc.tile_pool(name="y1p", bufs=2) as y1_pool,
        tc.tile_pool(name="outp", bufs=4) as out_pool,
        tc.tile_pool(name="ps", bufs=1, space="PSUM") as ps,
    ):
        # ---- resident constants (3 packed DMAs for fast startup) ----
        bn_w1_sb = consts.tile([128, S * 4 * BN], F8, tag="bn_w1")
        nc.sync.dma_start(bn_w1_sb[:], bn_w1t[:])
        bnc16_sb = consts.tile([128, nG * 128 + nG * GH + GH], MM_DT, tag="bnc16")
        bnc32_sb = consts.tile([128, 2 * nG + 2], F32, tag="bnc32")

        def load_bn_consts():
            nc.sync.dma_start(bnc32_sb[:], bnc32[:])
            nc.sync.dma_start(bnc16_sb[:], bnc16[:])
        bn_w2_sb = bnc16_sb[:, :nG * 128]
        gm_w1_sb = bnc16_sb[:, nG * 128:nG * 128 + nG * GH]
        gm_w2_sb = bnc16_sb[:, nG * 128 + nG * GH:]
        bn_b1_sb = bnc32_sb[:, :nG]
        bn_b2_sb = bnc32_sb[:, nG:2 * nG]
        gm_b1_sb = bnc32_sb[:, 2 * nG:2 * nG + 1]
        gm_b2_sb = bnc32_sb[:, 2 * nG + 1:]
        # fc-phase consts: allocate now, DMA deferred until after the gs8
        # chunk-0 stream so phase 1 starts sooner
        a_sb = consts.tile([128, BC], MM_DT, tag="aT")
        fcc_sb = consts.tile([128, 3 * S * 4], F32, tag="fcc32")
        d_sb = fcc_sb[:, :S * 4]
        b1_sb = fcc_sb[:, S * 4:2 * S * 4]
        b2_sb = fcc_sb[:, 2 * S * 4:]

        def load_fc_consts():
            nc.sync.dma_start(a_sb[:], aT[:])
            nc.sync.dma_start(fcc_sb[:], fcc32[:])

        gf2_sb = [consts.tile([GH, NB], MM_DT, tag=f"gf2c{c}", name=f"gf2c{c}")
                  for c in range(N_CHUNKS)]

        # PSUM tags: y1(2) + yo(4) + p1tmp(2) = 8 banks.

        def load_gs8_group(gi, c):
            """one DMA for a whole style group's fp8 gs chunk (ng segments)."""
            s0, ng = GROUPS[gi]
            t = gs8_pool.tile([128, ng * 4 * NB], F8,
                              tag="gs8a" if ng == 4 else "gs8b",
                              bufs=5 if ng == 4 else 2,
                              name=f"gs8_{gi}_{c}")
            nc.sync.dma_start(
                t[:].rearrange("p (s x) -> p s x", s=ng),
                gsT8c[c, s0:s0 + ng].rearrange("s p x -> p s x"))
            return t

        gf_tiles = {}      # (gi, c) -> SBUF gf tile

        def p1_bn1(gi, c, t, ps_tag):
            """bn first layer for one style group -> PSUM h1 accumulator."""
            s0, ng = GROUPS[gi]
            ps_h1 = ps.tile([128, NB], F32, tag=ps_tag,
                            bufs=(2 if ps_tag in ("p1tmp", "y1") else 1),
                            name=f"ps_h1_{gi}_{c}")
            for j in range(ng):
                for kt in range(4):
                    mm(ps_h1[32 * j:32 * j + 32, :],
                       bn_w1_sb[:, ((s0 + j) * 4 + kt) * BN:((s0 + j) * 4 + kt + 1) * BN],
                       t[:, (j * 4 + kt) * NB:(j * 4 + kt + 1) * NB],
                       start=(kt == 0), stop=(kt == 3),
                       tile_position=(0, 32 * j))
            return ps_h1

        def p1_h1act(gi, c, ps_h1, bufs=5):
            s0, ng = GROUPS[gi]
            pN = 32 * ng
            h1 = act1_pool.tile([128, NB], MM_DT, tag="h1s", bufs=bufs,
                                name=f"h1_{gi}_{c}")
            nc.scalar.activation(h1[:pN, :], ps_h1[:pN, :], Relu,
                                 bias=bn_b1_sb[:pN, gi:gi + 1], scale=1.0 / WSC)
            return h1

        def p1_bn2(gi, c, h1):
            s0, ng = GROUPS[gi]
            pN = 32 * ng
            ps_h2 = ps.tile([128, NB], F32, tag="p1tmp", bufs=2,
                            name=f"ps_h2_{gi}_{c}")
            mm(ps_h2[:pN, :], bn_w2_sb[:pN, gi * 128:gi * 128 + pN], h1[:pN, :])
            gf = act1_pool.tile([128, NB], MM_DT, tag=f"gf{gi}", bufs=2,
                                name=f"gf_{gi}_{c}")
            nc.scalar.activation(gf[:pN, :], ps_h2[:pN, :], Ident,
                                 bias=bn_b2_sb[:pN, gi:gi + 1])
            gf_tiles[(gi, c)] = gf

        def p1_group(gi, c, t):
            """serial-chain variant used for chunk 1 (hidden under fc work)."""
            ps_h1 = p1_bn1(gi, c, t, "p1tmp")
            if debug_taps and gi == 0 and c == 1:
                nc.gpsimd.dma_start(dbg["dbg_gs8"][:], t[:, :512])
            h1 = p1_h1act(gi, c, ps_h1)
            p1_bn2(gi, c, h1)

        def p1_gm(c):
            """global MLP for chunk c from the 5 gf tiles -> gf2_sb[c]."""
            ps_g1 = ps.tile([GH, NB], F32, tag="p1tmp", bufs=2, name=f"ps_g1_{c}")
            for gi, (s0, ng) in enumerate(GROUPS):
                pN = 32 * ng
                mm(ps_g1[:], gm_w1_sb[:pN, gi * GH:(gi + 1) * GH],
                   gf_tiles[(gi, c)][:pN, :],
                   start=(gi == 0), stop=(gi == len(GROUPS) - 1))
            gmh = act1_pool.tile([GH, NB], MM_DT, tag="gmh", bufs=2,
                                 name=f"gmh_{c}")
            nc.scalar.activation(gmh[:], ps_g1[:], Relu, bias=gm_b1_sb)
            ps_g2 = ps.tile([GH, NB], F32, tag="p1tmp", bufs=2, name=f"ps_g2_{c}")
            mm(ps_g2[:], gm_w2_sb, gmh[:])
            nc.scalar.activation(gf2_sb[c][:], ps_g2[:], Ident, bias=gm_b2_sb)
            if debug_taps:
                nc.gpsimd.dma_start(dbg["dbg_gf2"][c], gf2_sb[c][:])

        w_tiles = {}
        gs16_tiles = {}

        def pf_w(s):
            w = w_pool.tile([128, W1C + W2C], MM_DT, tag="w", bufs=7,
                            name=f"w_{s}")
            # two DMAs: >8KB per partition in one descriptor corrupts
            nc.sync.dma_start(w[:, :W1C], wpack[s, :, :W1C])
            nc.sync.dma_start(w[:, W1C:], wpack[s, :, W1C:])
            w_tiles[s] = w

        def pf_gs16(s, c):
            t = gs16_pool.tile([128, 4 * NB], MM_DT, tag="gs16", bufs=12,
                               name=f"gs16_{s}_{c}")
            nc.sync.dma_start(t[:], gsT16c[c, s])
            gs16_tiles[(s, c)] = t

        pending = []     # deferred epilogue of the previous fc block

        def flush_epilogue():
            while pending:
                ps_y, s, c, gs_sb, b0 = pending.pop(0)
                for dt_ in range(4):
                    o = out_pool.tile([128, NB], MM_DT, tag="o",
                                      name=f"o_{s}_{c}_{dt_}")
                    nc.vector.scalar_tensor_tensor(
                        o[:], ps_y[dt_][:], b2_sb[:, s * 4 + dt_:s * 4 + dt_ + 1],
                        gs_sb[:, dt_ * NB:(dt_ + 1) * NB], op0=ADD, op1=ADD)
                    nc.gpsimd.dma_start(
                        yT[s, dt_ * 128:(dt_ + 1) * 128, b0:b0 + NB], o[:])

        def fc1_open(s, c, ht):
            """start an fc1 accumulation with only the gs k-tiles (no gf2
            dependency) — used to fill PE idle before gf2[c] exists."""
            h0 = ht * 128
            ps_y1 = ps.tile([128, NB], F32, tag="y1", bufs=2,
                            name=f"ps_y1_{s}_{c}_{ht}")
            w = w_tiles[s]
            gs_sb = gs16_tiles[(s, c)]
            for kt in range(4):
                mm(ps_y1[:], w[:, kt * FCH + h0:kt * FCH + h0 + 128],
                   gs_sb[:, kt * NB:(kt + 1) * NB],
                   start=(kt == 0), stop=False)
            return ps_y1

        def fc_block(s, c, pre=None):
            b0 = c * NB
            w = w_tiles[s]
            gs_sb = gs16_tiles.pop((s, c))
            y1 = []
            for ht in range(4):
                h0 = ht * 128
                if pre and ht in pre:
                    ps_y1 = pre[ht]
                    mm(ps_y1[:], w[:, 4 * FCH + h0:4 * FCH + h0 + 128],
                       gf2_sb[c][:], start=False, stop=True)
                else:
                    ps_y1 = ps.tile([128, NB], F32, tag="y1", bufs=2,
                                    name=f"ps_y1_{s}_{c}_{ht}")
                    for kt in range(KT1):
                        mov = (gs_sb[:, kt * NB:(kt + 1) * NB] if kt < 4
                               else gf2_sb[c][:])
                        mm(ps_y1[:], w[:, kt * FCH + h0:kt * FCH + h0 + 128], mov,
                           start=(kt == 0), stop=(kt == KT1 - 1))
                # += age * d_s (rank-1 folded age-MLP contribution); stage in
                # SBUF so the DVE never writes PSUM (PE write-port contention)
                y1p = y1_pool.tile([128, NB], MM_DT, tag="y1pre", bufs=3,
                                   name=f"y1p_{s}_{c}_{ht}")
                nc.vector.scalar_tensor_tensor(
                    y1p[:], a_sb[:, b0:b0 + NB], d_sb[:, s * 4 + ht:s * 4 + ht + 1],
                    ps_y1[:], op0=MULT, op1=ADD)
                y1t = y1_pool.tile([128, NB], MM_DT, tag=f"y1_{ht}",
                                   name=f"y1_{s}_{c}_{ht}")
                nc.scalar.activation(y1t[:], y1p[:], Relu,
                                     bias=b1_sb[:, s * 4 + ht:s * 4 + ht + 1])
                y1.append(y1t)
                if debug_taps and s == 0 and c == 0 and ht == 0:
                    nc.gpsimd.dma_start(dbg["dbg_y1"][:], y1t[:])
                    nc.gpsimd.dma_start(dbg["dbg_a"][:], a_sb[:])
                    nc.gpsimd.dma_start(dbg["dbg_w"][:], w[:, :512])
                    nc.gpsimd.dma_start(dbg["dbg_gs16"][:], gs_sb[:, :512])
            # previous block's epilogue runs on the DVE during this block's
            # fc1 so the y1-stt chain is never queued behind it
            flush_epilogue()
            ps_y = [ps.tile([128, NB], F32, tag=f"yo{dt_}", name=f"ps_y_{s}_{c}_{dt_}")
                    for dt_ in range(4)]
            for kt in range(4):          # k-tile outer: no wait on the last ReLU
                for dt_ in range(4):
                    mm(ps_y[dt_][:],
                       w[:, W1C + (kt * 4 + dt_) * 128:W1C + (kt * 4 + dt_ + 1) * 128],
                       y1[kt][:],
                       start=(kt == 0), stop=(kt == 3))
            pending.append((ps_y, s, c, gs_sb, b0))

        # ---------------- emission ----------------
        # phase 1 chunk 0, cross-group pipelined: all bn1 GEMMs first
        # (borrowing the idle fc PSUM banks). ACT-queue order interleaves the
        # h1-ReLUs with the gf-ACTs so no engine's in-order queue waits on a
        # later DMA-paced group.
        p1c0_tags = ["p1tmp", "yo0", "yo1", "yo2", "yo3"]
        ps_h1s = [None] * len(GROUPS)
        h1s = [None] * len(GROUPS)
        t0 = load_gs8_group(0, 0)
        t1 = load_gs8_group(1, 0)
        load_bn_consts()
        # HAM warmup: ~10 back-to-back matmuls into a scratch PSUM bank as
        # soon as the first weight tile lands; sustains the PE-busy window so
        # the clock is at 2.4 GHz before real phase-1 work, which otherwise
        # runs entirely in the cold 1.2 GHz state (it is DMA-paced and never
        # stays busy long enough to warm up on its own)
        warm_ps = ps.tile([128, NB], F32, tag="yo3", name="warm_ps")

        def warm(n):
            # dependency-free matmuls: run only while real work waits on DMA,
            # keeping the HAM clock at 2.4 GHz through the DMA-paced prologue.
            # All warm mms must be emitted before group 4's p1_bn1 (it reuses
            # the yo3 bank warm_ps writes to).
            for _ in range(n):
                mm(warm_ps[:], bn_w1_sb[:, :128], bn_w1_sb[:, :NB])

        # NOTE: more warmup (e.g. filling every phase-1 DMA gap with dummies)
        # trips the chip's sustained-power P0 downclock — the whole run then
        # executes at 2.0 GHz instead of 2.4 (measured 408us vs 338us).
        warm(10)
        ps_h1s[0] = p1_bn1(0, 0, t0, p1c0_tags[0])
        ps_h1s[1] = p1_bn1(1, 0, t1, p1c0_tags[1])
        h1s[0] = p1_h1act(0, 0, ps_h1s[0], bufs=5)
        for gi in range(2, len(GROUPS)):
            t = load_gs8_group(gi, 0)
            ps_h1s[gi] = p1_bn1(gi, 0, t, p1c0_tags[gi])
            h1s[gi - 1] = p1_h1act(gi - 1, 0, ps_h1s[gi - 1], bufs=5)
            p1_bn2(gi - 2, 0, h1s[gi - 2])
        h1s[-1] = p1_h1act(len(GROUPS) - 1, 0, ps_h1s[-1], bufs=5)
        pf_w(0)
        pf_gs16(0, 0)
        for gi in range(len(GROUPS) - 2, len(GROUPS)):
            p1_bn2(gi, 0, h1s[gi])
        # pre-open fc(0,0)'s first two fc1 accumulations (gs k-tiles only) to
        # fill the PE idle under the global-MLP ACT chain
        pre00 = {0: fc1_open(0, 0, 0), 1: fc1_open(0, 0, 1)}
        p1_gm(0)
        load_fc_consts()

        # start region: interleave fc group-0 deps with the chunk-1 gs8
        # stream; all of these DMA issues are wait-free so arrival order on
        # the sync queue == issue order == need order.
        gs8_c1 = {}
        for gi in range(len(GROUPS)):
            if 1 <= gi < 4:
                pf_w(gi)
                pf_gs16(gi, 0)
            gs8_c1[gi] = load_gs8_group(gi, 1)

        # fc chunk 0 of group 0, interleaved with phase-1 chunk 1 in stages
        # (bn1 / ReLU emitted one fc block before bn2 so the in-order tensor
        # queue never waits on an ACT chain)
        pc1 = {}
        hc1 = {}

        def p1c1_ab(gi):
            pc1[gi] = p1_bn1(gi, 1, gs8_c1[gi], "p1tmp")
            hc1[gi] = p1_h1act(gi, 1, pc1[gi])

        fc_block(0, 0, pre=pre00)
        p1c1_ab(0)
        pf_gs16(4, 0)
        pf_w(4)
        fc_block(1, 0)
        p1_bn2(0, 1, hc1[0])
        p1c1_ab(1)
        pf_gs16(5, 0)
        pf_w(5)
        fc_block(2, 0)
        p1_bn2(1, 1, hc1[1])
        p1c1_ab(2)
        # group-0 chunk-1 tiles are not needed until after fc(5,0); issuing
        # them here keeps the saturated startup DMA window free for the
        # pulled-forward fc(4,0)/fc(5,0) dependencies
        for s in range(4):
            pf_gs16(s, 1)
        fc_block(3, 0)
        p1_bn2(2, 1, hc1[2])
        p1c1_ab(3)
        # two group-1 chunk-0 blocks pulled forward so the phase-1 chunk-1
        # tail (g3/g4 chains + global MLP) hides under fc work instead of
        # stalling fc(0,1) on the gf2[1] ACT chain
        fc_block(4, 0)
        p1_bn2(3, 1, hc1[3])
        p1c1_ab(4)
        fc_block(5, 0)
        p1_bn2(4, 1, hc1[4])
        p1_gm(1)

        # fc chunk 1 of group 0, prefetching the rest of group 1
        pf_gs16(6, 0)
        pf_gs16(7, 0)
        pf_w(6)
        fc_block(0, 1)
        pf_w(7)
        fc_block(1, 1)
        for s in range(4, 8):
            pf_gs16(s, 1)
        # group 2's chunk-0 deps go out early: group 1's own chunk-0 list is
        # only two blocks long, too late for a just-in-time prefetch there
        for s in range(8, 12):
            pf_gs16(s, 0)
        pf_w(8)
        fc_block(2, 1)
        fc_block(3, 1)

        # steady groups: chunk 0 then chunk 1, prefetching group g+1 at
        # points where the buffer rotation never blocks the sync queue
        for g in range(1, len(GROUPS)):
            s0, ng = GROUPS[g]
            nxt = GROUPS[g + 1] if g + 1 < len(GROUPS) else None
            c0_styles = [6, 7] if g == 1 else list(range(s0, s0 + ng))
            for bi, s in enumerate(c0_styles):
                if bi == 0 and nxt:
                    if g == 1:
                        pf_w(9)      # rest of group 2 prefetched earlier
                    else:
                        for s2 in range(nxt[0], nxt[0] + nxt[1]):
                            pf_gs16(s2, 0)
                        for s2 in range(nxt[0], min(nxt[0] + 2, nxt[0] + nxt[1])):
                            pf_w(s2)
                fc_block(s, 0)
            for bi, s in enumerate(range(s0, s0 + ng)):
                if bi == 0 and nxt:
                    for s2 in range(nxt[0], nxt[0] + nxt[1]):
                        pf_gs16(s2, 1)
                if bi == 1 and nxt:
                    for s2 in range(nxt[0] + 2, nxt[0] + nxt[1]):
                        pf_w(s2)
                fc_block(s, 1)
        flush_epilogue()

    nc.compile()
    return nc


def _prep_weights(bn_w1, bn_b1, bn_w2, bn_b2, gm_w1, gm_b1, gm_w2, gm_b2,
                  age_w1, age_w2, fc_w1, fc_b1, fc_w2, fc_b2):
    import ml_dtypes
    f = np.float32
    h = NP_MM
    f8 = ml_dtypes.float8_e4m3
    nG = len(GROUPS)
    # [p, (s, kt, j)] : bn_w1[s, kt*128+p, j], pre-scaled for fp8
    bn_w1t = np.ascontiguousarray(
        (bn_w1 * WSC).reshape(S, 4, 128, BN).transpose(2, 0, 1, 3)
        .reshape(128, S * 4 * BN).astype(f8))
    bn_b1g = np.zeros((128, nG), f)
    bn_b2g = np.zeros((128, nG), f)
    bn_w2bd = np.zeros((128, nG * 128), h)
    for gi, (s0, ng) in enumerate(GROUPS):
        for j in range(ng):
            bn_b1g[32 * j:32 * j + 32, gi] = bn_b1[s0 + j]
            bn_b2g[32 * j:32 * j + 32, gi] = bn_b2[s0 + j]
            bn_w2bd[32 * j:32 * j + 32, gi * 128 + 32 * j:gi * 128 + 32 * j + 32] = bn_w2[s0 + j]
    gm_w1p = np.zeros((nG * 128, GH), f)
    gm_w1p[:S * BN] = gm_w1
    gm_w1g = np.ascontiguousarray(
        gm_w1p.reshape(nG, 128, GH).transpose(1, 0, 2).reshape(128, nG * GH), h)
    # fc1 rows reordered to [gs (512) | gf (128)]; age handled as rank-1
    w1p = np.empty((S, KT1 * 128, FCH), h)
    w1p[:, :4 * 128] = fc_w1[:, GH + AH:]
    w1p[:, 4 * 128:] = fc_w1[:, :GH]
    fc_w1t = w1p.reshape(S, KT1, 128, FCH).transpose(0, 2, 1, 3).reshape(S, 128, W1C)
    fc_w2t = fc_w2.reshape(S, 4, 128, 4, 128).transpose(0, 2, 1, 3, 4).reshape(S, 128, W2C).astype(h)
    wpack = np.ascontiguousarray(np.concatenate([fc_w1t, fc_w2t], axis=2))
    # rank-1 age direction: d[s, h] = sum_k relu(age_w1)[0,k'] age_w2[k',k] fc_w1[s, GH+k, h]
    c16 = (np.maximum(age_w1[0].astype(np.float64), 0) @ age_w2.astype(np.float64))
    d = np.einsum('k,skh->sh', c16, fc_w1[:, GH:GH + AH, :].astype(np.float64))
    dT = d.reshape(S, 4, 128).transpose(2, 0, 1).reshape(128, S * 4)
    b1T = fc_b1.reshape(S, 4, 128).transpose(2, 0, 1).reshape(128, S * 4)
    b2T = fc_b2.reshape(S, 4, 128).transpose(2, 0, 1).reshape(128, S * 4)
    fcc32 = np.ascontiguousarray(
        np.concatenate([dT, b1T, b2T], axis=1).astype(f))
    bnc16 = np.ascontiguousarray(
        np.concatenate([bn_w2bd, gm_w1g, gm_w2.astype(h)], axis=1))
    bnc32 = np.ascontiguousarray(np.concatenate(
        [bn_b1g, bn_b2g, gm_b1.reshape(GH, 1).astype(f),
         gm_b2.reshape(GH, 1).astype(f)], axis=1).astype(f))
    return dict(bn_w1t=bn_w1t, bnc16=bnc16, bnc32=bnc32, wpack=wpack,
                fcc32=fcc32)


def run(inputs: dict, trace: bool = False):
    """Build in_maps from full inputs, run SPMD on 8 cores, return
    (full_output, BassKernelResults)."""
    import ml_dtypes
    if "nc" not in _CACHE:
        _CACHE["nc"] = build_program()
    nc = _CACHE["nc"]

    gs = inputs["global_styles"]
    ages = inputs["target_ages"]
    w = _prep_weights(
        inputs["bn_w1"], inputs["bn_b1"], inputs["bn_w2"], inputs["bn_b2"],
        inputs["gm_w1"], inputs["gm_b1"], inputs["gm_w2"], inputs["gm_b2"],
        inputs["age_w1"], inputs["age_w2"],
        inputs["fc_w1"], inputs["fc_b1"], inputs["fc_w2"], inputs["fc_b2"])

    gsT_full = np.ascontiguousarray(gs.transpose(1, 2, 0).astype(NP_MM))  # [S, D, B]
    a16 = ages.astype(NP_MM)
    in_maps = []
    for core in range(N_CORES):
        sl = slice(core * BC, (core + 1) * BC)
        # pre-chunked contiguous layout: [c, s, p, kt*NB+b]
        g16c = np.ascontiguousarray(
            gsT_full[:, :, sl].reshape(S, 4, 128, N_CHUNKS, NB)
            .transpose(3, 0, 2, 1, 4).reshape(N_CHUNKS, S, 128, 4 * NB))
        m = dict(w)
        m["gsT16c"] = g16c
        m["gsT8c"] = g16c.astype(ml_dtypes.float8_e4m3)
        m["aT"] = np.ascontiguousarray(np.broadcast_to(a16[None, sl], (128, BC)))
        in_maps.append(m)

    res = run_bass_kernel_spmd(nc, in_maps, core_ids=list(range(N_CORES)),
                               trace=trace)
    yT = np.concatenate([res.results[c]["yT"][:, :, :] for c in range(N_CORES)],
                        axis=2)                              # [S, D, B] fp16
    y = np.ascontiguousarray(yT.transpose(2, 0, 1), np.float32)  # [B, S, D]
    return y, res


def kernel(**inputs) -> np.ndarray:
    y, _ = run(inputs, trace=False)
    return y
